# revision 4
# baseline (speedup 1.0000x reference)
"""Trainium2 Bass kernel for the GPCwSTU rollout (nn_GPCwSTU_72576357368005).

Math restructure: the sequential rollout is the lower-triangular linear system
    u_t = d_t - sum_{s<t} K (Ecat^T (phi_s (x) u_s)),
with d_t = bias + sum_i E[:,:,i] @ w_{t-4+i} precomputable in parallel.
The coupling is weak, so Richardson iteration with EXACT cross-core offsets
    u <- d - prefix_t( K Ecat^T (phi (x) u) )
reaches 1.5e-3 loss error in NITERS=2 iterations (validated in fp64/quantized
numpy emulation against the sequential reference; gate is 2e-2).

Differences from the 720us baseline:
  - two-step contraction y = K (Ecat^T O) instead of y = F^T O: kills the
    137us replicated Fmat=Ecat@K^T precompute entirely.
  - fp8 (e4m3, DoubleRow) for the big Ecat^T O contraction: 2x tensor rate,
    and the 10.5MB scaled Ecat stays resident in SBUF. Quantization noise
    averages out in the quadratic loss (adds <1e-4).
  - exact (same-iteration) offsets instead of lagged ones: 2 iterations
    replace 6; per-iteration AllGather of 2KB block sums.
  - the final X-offset AllGather overlaps with Q*Xlocal / R*u matmuls; the
    offset enters as a per-partition scalar fixup afterwards.

Scale bookkeeping: Ecat is scaled by S_E and phi by S_O for fp8 range; the
z / X values on device are scaled by S_E*S_O. K^T is pre-divided by S_E*S_O
(so y and u are true-scale), Q by (S_E*S_O)^2 (so X*(QX) is true-scale).

Layouts are feature-major ([feature, t]); t is sharded 256 steps/core.
"""

import sys

sys.path.insert(0, "/opt/trn_rl_repo")

import numpy as np
import ml_dtypes

import concourse.bass as bass
import concourse.bacc as bacc
import concourse.mybir as mybir
from concourse import tile
from concourse.bass_utils import run_bass_kernel_spmd

BF16 = mybir.dt.bfloat16
F32 = mybir.dt.float32
FP8 = mybir.dt.float8e4
AL = mybir.AluOpType
ACT = mybir.ActivationFunctionType
DR = mybir.MatmulPerfMode.DoubleRow

T, N, MC, KF, M = 2048, 1024, 512, 20, 5
NCORES = 8
TS = T // NCORES          # 256 timesteps per core
NK = N // 128             # 8 tiles over state dim
CT = MC // 128            # 4 tiles over control dim
ICT = (KF * MC) // 128    # 80 tiles over the (filter, control) contraction
NITERS = 2                # exact-offset Richardson iterations

S_E = 4096.0              # fp8 scale on Ecat
S_O = 64.0                # fp8 scale on O = phi (x) u  (folded into phiB)

_CACHE = {}


def build_nc(debug=False, reps=1):
    nc = bacc.Bacc(None, target_bir_lowering=False, debug=False)

    # ---- I/O ----
    wT_d = nc.declare_dram_parameter("wT", [N, TS + M - 1], BF16, isOutput=False)
    ET_d = nc.declare_dram_parameter("ET", [M, N, MC], BF16, isOutput=False)
    Ecat8_d = nc.declare_dram_parameter("Ecat8", [KF * MC, N], FP8, isOutput=False)
    KT_d = nc.declare_dram_parameter("KT", [N, MC], BF16, isOutput=False)
    Q_d = nc.declare_dram_parameter("Q", [N, N], BF16, isOutput=False)
    R_d = nc.declare_dram_parameter("R", [MC, MC], BF16, isOutput=False)
    phiB_d = nc.declare_dram_parameter("phiB", [128, KF, TS], BF16, isOutput=False)
    biasT_d = nc.declare_dram_parameter("biasT", [MC, 1], F32, isOutput=False)
    mask_d = nc.declare_dram_parameter("mask", [NCORES, 1], F32, isOutput=False)
    loss_d = nc.declare_dram_parameter("loss", [1, TS], F32, isOutput=True)
    if debug:
        dbg_d = nc.declare_dram_parameter("dbg_d", [128, CT, TS], F32, isOutput=True)
        dbg_u = nc.declare_dram_parameter("dbg_u", [128, CT, TS], F32, isOutput=True)
        dbg_z = nc.declare_dram_parameter("dbg_z", [128, NK, TS], F32, isOutput=True)
        dbg_X = nc.declare_dram_parameter("dbg_X", [128, NK, TS], F32, isOutput=True)

    # collective bounce buffers
    bsum_d = nc.dram_tensor("bsum", [MC], F32)
    bgat_d = nc.dram_tensor("bgat", [NCORES, MC], F32, addr_space="Shared")
    bxsum_d = nc.dram_tensor("bxsum", [N], F32)
    bxgat_d = nc.dram_tensor("bxgat", [NCORES, N], F32, addr_space="Shared")

    with tile.TileContext(nc) as tc:
        with (
            tc.tile_pool(name="const", bufs=1) as cpool,
            tc.tile_pool(name="live", bufs=1) as opool,
            tc.tile_pool(name="work", bufs=2) as wpool,
        ):
            # ---- constants / resident weights (gpsimd queue: parallel with
            # the sync-queue phase-1 loads) ----
            phiB = cpool.tile([128, KF, TS], BF16)
            nc.gpsimd.dma_start(phiB[:], phiB_d[:])
            KTs = cpool.tile([128, NK, MC], BF16)
            nc.gpsimd.dma_start(KTs[:], KT_d.ap().rearrange("(k p) c -> p k c", p=128))
            Rs = cpool.tile([128, CT, MC], BF16)
            nc.gpsimd.dma_start(Rs[:], R_d.ap().rearrange("(k p) c -> p k c", p=128))
            biasT = cpool.tile([128, CT, 1], F32)
            nc.gpsimd.dma_start(biasT[:], biasT_d.ap().rearrange("(c p) one -> p c one", p=128))
            mask = cpool.tile([NCORES, 1], F32)
            nc.gpsimd.dma_start(mask[:], mask_d[:])
            zeros = cpool.tile([128, TS], F32)
            nc.vector.memset(zeros[:], 0.0)
            ones = cpool.tile([128, 1], F32)
            nc.vector.memset(ones[:], 1.0)

            for rep in range(reps):
                # ---- long-lived state ----
                d = opool.tile([128, CT, TS], F32)
                ubf = opool.tile([128, CT, TS], BF16)
                a = opool.tile([128, CT, TS], F32)
                O8 = opool.tile([128, KF, CT, TS], FP8)
                zsb = opool.tile([128, NK, TS], BF16)
                ysb = opool.tile([128, CT, TS], F32)
                Ecat8 = opool.tile([128, ICT, N], FP8)
                Qs = opool.tile([128, NK, N], BF16)
                Xbf = opool.tile([128, NK, TS], BF16)
                offS = opool.tile([128, CT, 1], F32)
                Bloc = opool.tile([128, CT, 1], F32)

                # ---- phase 1: d = bias + sum_i E_i @ w_shift_i (bf16) ----
                with (
                    tc.tile_pool(name="p1", bufs=1) as p1,
                    tc.tile_pool(name="p1s", bufs=2) as p1s,
                    tc.tile_pool(name="p1ps", bufs=1, space="PSUM") as p1ps,
                ):
                    wTs = p1.tile([128, NK, TS + M - 1], BF16)
                    nc.sync.dma_start(wTs[:], wT_d.ap().rearrange("(k p) t -> p k t", p=128))
                    dps = p1ps.tile([128, CT, TS], F32)
                    for i in range(M):
                        ETs = p1s.tile([128, NK, MC], BF16, tag="et")
                        nc.sync.dma_start(ETs[:], ET_d[i].rearrange("(k p) c -> p k c", p=128))
                        for k in range(NK):
                            for ct in range(CT):
                                # one start per PSUM bank (2 regions/bank)
                                nc.tensor.matmul(
                                    dps[:, ct, :],
                                    ETs[:, k, ct * 128:(ct + 1) * 128],
                                    wTs[:, k, i:i + TS],
                                    start=(i == 0 and k == 0 and ct % 2 == 0),
                                    stop=(i == M - 1 and k == NK - 1),
                                )
                    # resident fp8 Ecat, per-kf chunks so z matmuls start early
                    for kf in range(KF):
                        nc.gpsimd.dma_start(
                            Ecat8[:, kf * CT:(kf + 1) * CT, :],
                            Ecat8_d[kf * MC:(kf + 1) * MC, :].rearrange("(k p) n -> p k n", p=128),
                        )
                    for ct in range(CT):
                        nc.scalar.activation(d[:, ct, :], dps[:, ct, :], ACT.Identity,
                                             bias=biasT[:, ct, :], scale=1.0)
                        nc.vector.tensor_copy(ubf[:, ct, :], dps[:, ct, :])
                    if debug and rep == 0:
                        nc.sync.dma_start(dbg_d[:], d[:])

                # deferred big weights (needed only in the loss phase)
                nc.gpsimd.dma_start(Qs[:], Q_d.ap().rearrange("(k p) n -> p k n", p=128))

                with (
                    tc.tile_pool(name="zps_p", bufs=1, space="PSUM") as zps_p,
                    tc.tile_pool(name="yps_p", bufs=1, space="PSUM") as yps_p,
                    tc.tile_pool(name="off_p", bufs=1, space="PSUM") as off_p,
                ):
                    zps = zps_p.tile([128, NK, TS], F32)
                    yps = yps_p.tile([128, CT, TS], F32)
                    offp = off_p.tile([128, CT, 1], F32)

                    for it in range(NITERS + 1):
                        last = it == NITERS
                        # O = phi (x) u, per-kf chunks pipelined into matmuls
                        for kf in range(KF):
                            nc.vector.tensor_tensor(
                                O8[:, kf, :, :], ubf[:, :, :],
                                phiB[:, kf, :].unsqueeze(1).broadcast_to([128, CT, TS]),
                                op=AL.mult,
                            )
                            # z += Ecat8_kf^T O_kf (fp8 DoubleRow: 256-row steps)
                            for h in range(2):
                                kk = kf * CT + h * 2
                                for nt in range(NK):
                                    nc.tensor.matmul(
                                        zps[:, nt, :],
                                        Ecat8[:, kk:kk + 2, nt * 128:(nt + 1) * 128],
                                        O8[:, kf, h * 2:h * 2 + 2, :],
                                        start=(kf == 0 and h == 0 and nt % 2 == 0),
                                        stop=(kf == KF - 1 and h == 1),
                                        perf_mode=DR,
                                    )
                        if not last:
                            # y = K z ; block sums ; AllGather ; exact offsets
                            for nt in range(NK):
                                nc.scalar.copy(zsb[:, nt, :], zps[:, nt, :])
                            for ct in range(CT):
                                for k in range(NK):
                                    nc.tensor.matmul(
                                        yps[:, ct, :],
                                        KTs[:, k, ct * 128:(ct + 1) * 128],
                                        zsb[:, k, :],
                                        start=(k == 0 and ct % 2 == 0),
                                        stop=(k == NK - 1),
                                    )
                            for ct in range(CT):
                                nc.scalar.activation(ysb[:, ct, :], yps[:, ct, :], ACT.Identity,
                                                     bias=0.0, scale=1.0,
                                                     accum_out=Bloc[:, ct, :])
                                nc.sync.dma_start(bsum_d[ct * 128:(ct + 1) * 128], Bloc[:, ct, :])
                            nc.gpsimd.collective_compute(
                                "AllGather", AL.bypass,
                                ins=[bsum_d[:]], outs=[bgat_d[:]],
                                replica_groups=[list(range(NCORES))],
                            )
                            gat = wpool.tile([NCORES, MC], F32, tag="gat")
                            nc.gpsimd.dma_start(gat[:], bgat_d[:])
                            for ct in range(CT):
                                nc.tensor.matmul(
                                    offp[:, ct, :], gat[:, ct * 128:(ct + 1) * 128], mask[:],
                                    start=(ct == 0), stop=True,
                                )
                            for ct in range(CT):
                                nc.scalar.copy(offS[:, ct, :], offp[:, ct, :])
                            for ct in range(CT):
                                nc.vector.tensor_copy(a[:, ct, 0:1], offS[:, ct, :])
                                nc.vector.tensor_tensor_scan(
                                    a[:, ct, 1:TS], ysb[:, ct, 0:TS - 1], zeros[:, 0:TS - 1],
                                    offS[:, ct, :], op0=AL.add, op1=AL.add,
                                )
                            nc.vector.tensor_tensor(ubf[:, :, :], d[:, :, :], a[:, :, :],
                                                    op=AL.subtract)
                            if debug and rep == 0 and it == NITERS - 1:
                                ud = wpool.tile([128, CT, TS], F32, tag="ud")
                                nc.vector.tensor_tensor(ud[:], d[:], a[:], op=AL.subtract)
                                nc.sync.dma_start(dbg_u[:], ud[:])
                        else:
                            # final pass: X offsets via AllGather of z col sums
                            BX = wpool.tile([128, NK, 1], F32, tag="bx")
                            Xp = opool.tile([128, NK, TS], F32)
                            for nt in range(NK):
                                nc.scalar.activation(Xp[:, nt, :], zps[:, nt, :], ACT.Identity,
                                                     bias=0.0, scale=1.0,
                                                     accum_out=BX[:, nt, :])
                                nc.sync.dma_start(bxsum_d[nt * 128:(nt + 1) * 128], BX[:, nt, :])
                            nc.gpsimd.collective_compute(
                                "AllGather", AL.bypass,
                                ins=[bxsum_d[:]], outs=[bxgat_d[:]],
                                replica_groups=[list(range(NCORES))],
                            )
                            gatx = wpool.tile([NCORES, N], F32, tag="gatx")
                            nc.gpsimd.dma_start(gatx[:], bxgat_d[:])
                            # local X prefix (zero offset) while the AllGather flies
                            for nt in range(NK):
                                nc.vector.memset(Xbf[:, nt, 0:1], 0.0)
                                nc.vector.tensor_tensor_scan(
                                    Xbf[:, nt, 1:TS], Xp[:, nt, 0:TS - 1], zeros[:, 0:TS - 1],
                                    0.0, op0=AL.add, op1=AL.add,
                                )
                            if debug and rep == 0:
                                nc.sync.dma_start(dbg_z[:], Xp[:])

                # ---- loss = sum_n X(QX) + sum_c u(Ru), with X-offset fixup ----
                with (
                    tc.tile_pool(name="p5ps", bufs=1, space="PSUM") as p5ps,
                    tc.tile_pool(name="p5", bufs=1) as p5,
                ):
                    qxps = p5ps.tile([128, NK, TS], F32)
                    rups = p5ps.tile([128, CT, TS], F32)
                    ofq = p5ps.tile([128, 2, NK, 1], F32)  # [0]=offX, [1]=Q offX
                    # R u and Q Xlocal run during the X-AllGather
                    for ct in range(CT):
                        for k in range(CT):
                            nc.tensor.matmul(
                                rups[:, ct, :], Rs[:, k, ct * 128:(ct + 1) * 128], ubf[:, k, :],
                                start=(k == 0 and ct % 2 == 0), stop=(k == CT - 1),
                            )
                    for nt in range(NK):
                        for k in range(NK):
                            nc.tensor.matmul(
                                qxps[:, nt, :], Qs[:, k, nt * 128:(nt + 1) * 128], Xbf[:, k, :],
                                start=(k == 0 and nt % 2 == 0), stop=(k == NK - 1),
                            )
                    offxS = p5.tile([128, NK, 1], F32)
                    offxB = p5.tile([128, NK, 1], BF16)
                    qoffS = p5.tile([128, NK, 1], F32)
                    for nt in range(NK):
                        nc.tensor.matmul(
                            ofq[:, 0, nt, :], gatx[:, nt * 128:(nt + 1) * 128], mask[:],
                            start=(nt == 0), stop=True,
                        )
                    for nt in range(NK):
                        nc.scalar.copy(offxS[:, nt, :], ofq[:, 0, nt, :])
                        nc.scalar.copy(offxB[:, nt, :], ofq[:, 0, nt, :])
                    for nt in range(NK):
                        for k in range(NK):
                            nc.tensor.matmul(
                                ofq[:, 1, nt, :], Qs[:, k, nt * 128:(nt + 1) * 128], offxB[:, k, :],
                                start=False, stop=(k == NK - 1),
                            )
                    for nt in range(NK):
                        nc.scalar.copy(qoffS[:, nt, :], ofq[:, 1, nt, :])
                    # true X = Xlocal + offX ; prod = X * (QXlocal + Q offX)
                    prod = p5.tile([128, NK, TS], F32)
                    prodr = p5.tile([128, CT, TS], F32)
                    for nt in range(NK):
                        nc.vector.tensor_scalar_add(Xbf[:, nt, :], Xbf[:, nt, :], offxS[:, nt, :])
                    if debug and rep == 0:
                        xd = wpool.tile([128, NK, TS], F32, tag="xd")
                        for nt in range(NK):
                            nc.vector.tensor_copy(xd[:, nt, :], Xbf[:, nt, :])
                        nc.sync.dma_start(dbg_X[:], xd[:])
                    for nt in range(NK):
                        nc.vector.scalar_tensor_tensor(
                            prod[:, nt, :], qxps[:, nt, :], qoffS[:, nt, :], Xbf[:, nt, :],
                            op0=AL.add, op1=AL.mult,
                        )
                    for ct in range(CT):
                        nc.vector.tensor_tensor(prodr[:, ct, :], ubf[:, ct, :], rups[:, ct, :],
                                                op=AL.mult)
                    with tc.tile_pool(name="lpsp", bufs=1, space="PSUM") as lpsp:
                        lps = lpsp.tile([1, TS], F32)
                        for nt in range(NK):
                            nc.tensor.matmul(lps[:], ones[:], prod[:, nt, :],
                                             start=(nt == 0), stop=False)
                        for ct in range(CT):
                            nc.tensor.matmul(lps[:], ones[:], prodr[:, ct, :],
                                             start=False, stop=(ct == CT - 1))
                        loss = wpool.tile([1, TS], F32, tag="loss")
                        nc.vector.tensor_copy(loss[:], lps[:])
                        nc.sync.dma_start(loss_d[:], loss[:])

    nc.compile()
    return nc


def _prep_inputs(inputs):
    f32 = np.float32
    bf = ml_dtypes.bfloat16
    f8 = ml_dtypes.float8_e4m3
    E = np.asarray(inputs["E"], f32)            # [MC, N, M]
    K = np.asarray(inputs["K"], f32)            # [MC, N]
    E_stu = np.asarray(inputs["E_stu"], f32)    # [KF, MC, N]
    phi = np.asarray(inputs["phi"], f32)        # [T, KF]
    w = np.asarray(inputs["w_test"], f32)       # [T, N]
    Q = np.asarray(inputs["Q"], f32)
    R = np.asarray(inputs["R"], f32)
    bias = np.asarray(inputs["bias"], f32)

    ET = np.ascontiguousarray(E.transpose(2, 1, 0)).astype(bf)   # [M, N, MC]
    Ecat = E_stu.reshape(KF * MC, N)
    Ecat8 = np.clip(Ecat * S_E, -240, 240).astype(f8)
    KTb = np.ascontiguousarray(K.T / (S_E * S_O)).astype(bf)
    Qb = (Q / (S_E * S_O) ** 2).astype(bf)
    Rb = R.astype(bf)
    biasT = np.ascontiguousarray(bias[:, None]).astype(f32)
    # w^T padded with M-1 zero columns at the left (for t<0 history)
    wTp = np.concatenate([np.zeros((N, M - 1), f32), np.ascontiguousarray(w.T)], axis=1)
    phiT = np.ascontiguousarray(phi.T) * S_O                      # [KF, T]

    in_maps = []
    for r in range(NCORES):
        t0 = r * TS
        wT_r = np.ascontiguousarray(wTp[:, t0:t0 + TS + M - 1]).astype(bf)
        phiB_r = np.broadcast_to(
            phiT[None, :, t0:t0 + TS], (128, KF, TS)
        ).astype(bf)
        mask_r = np.zeros((NCORES, 1), f32)
        mask_r[:r] = 1.0
        in_maps.append({
            "wT": wT_r, "ET": ET, "Ecat8": Ecat8, "KT": KTb,
            "Q": Qb, "R": Rb, "phiB": np.ascontiguousarray(phiB_r),
            "biasT": biasT, "mask": mask_r,
        })
    return in_maps


def kernel(**inputs) -> np.ndarray:
    if "nc" not in _CACHE:
        _CACHE["nc"] = build_nc()
    nc = _CACHE["nc"]
    in_maps = _prep_inputs(inputs)
    res = run_bass_kernel_spmd(nc, in_maps, list(range(NCORES)))
    out = np.concatenate([res.results[r]["loss"][0] for r in range(NCORES)])
    return out.astype(np.float32)
